# revision 2
# baseline (speedup 1.0000x reference)
"""Trainium2 Bass kernel for the attention-pooling layer.

Math per batch b (reference):
    h      = X[b, -1, :]                    # [F]
    score  = tanh(X[b] @ W_o + h @ W_h + b) # [T, U]
    logits = score @ v                      # [T]
    w      = softmax(logits)                # [T]
    ctx    = w @ X[b]                       # [F]
returns (ctx [B, F], w [B, T]).

Device strategy (data-parallel over batch, 8 cores x 32 batches):
  - X cast-loaded f32->bf16 (SWDGE DMA cast) in natural [T, F] tile layout.
  - X.T produced by the DMA xbar transpose (bf16, SBUF->SBUF).
  - score.T [U, T] on TensorE: lhsT = W_o tile (stationary), rhs = X.T.
    This layout makes (h @ W_h + b) a per-partition scalar, so bias+tanh
    is a single ScalarE activation per tile.
  - logits as [128t, 8] tiles: lhsT = score.T slice, rhs = v.
  - e = exp(logits) unshifted: |logits| <= sum|v| ~ 20, far from overflow,
    and softmax is shift-invariant, so this matches the reference exactly
    up to fp rounding.
  - ctx_unnorm = e.T @ X (bf16).  Normalization (divide by sum e) on host.
"""

import numpy as np

import concourse.bass as bass
import concourse.tile as tile
from concourse import bacc, mybir
from concourse.bass_utils import run_bass_kernel_spmd
from concourse.masks import make_identity

B, T, F, U = 256, 1024, 512, 256
N_CORES = 8
BPC = B // N_CORES  # batches per core

FP32 = mybir.dt.float32
BF16 = mybir.dt.bfloat16

NT = T // 128  # 8 t-tiles
NF = F // 128  # 4 f-tiles
NU = U // 128  # 2 u-tiles


def build_nc():
    nc = bacc.Bacc("TRN2", target_bir_lowering=False, debug=False)

    x_d = nc.dram_tensor("x", [BPC, T, F], FP32, kind="ExternalInput").ap()
    wo_d = nc.dram_tensor("w_o", [F, U], FP32, kind="ExternalInput").ap()
    wh_d = nc.dram_tensor("w_h", [F, U], FP32, kind="ExternalInput").ap()
    b_d = nc.dram_tensor("b", [U], FP32, kind="ExternalInput").ap()
    v_d = nc.dram_tensor("v", [U], FP32, kind="ExternalInput").ap()

    e_d = nc.dram_tensor("e_out", [BPC, T], FP32, kind="ExternalOutput").ap()
    ctx_d = nc.dram_tensor("ctx_out", [BPC, F], FP32, kind="ExternalOutput").ap()

    Tanh = mybir.ActivationFunctionType.Tanh
    Exp = mybir.ActivationFunctionType.Exp
    Ident = mybir.ActivationFunctionType.Identity

    with tile.TileContext(nc) as tc:
        with (
            tc.tile_pool(name="const", bufs=1) as const,
            tc.tile_pool(name="xpool", bufs=2) as xpool,
            tc.tile_pool(name="xtpool", bufs=2) as xtpool,
            tc.tile_pool(name="scpool", bufs=4) as scpool,
            tc.tile_pool(name="smsb", bufs=2) as smsb,
            tc.tile_pool(name="pspool", bufs=4, space="PSUM") as pspool,
            tc.tile_pool(name="psmall", bufs=1, space="PSUM") as psmall,
        ):
            # ---- constants / preamble ----
            wo_bf = const.tile([128, NF, U], BF16)
            nc.gpsimd.dma_start(
                out=wo_bf[:], in_=wo_d.rearrange("(fi p) u -> p fi u", p=128)
            )
            wh_bf = const.tile([128, NF, U], BF16)
            nc.gpsimd.dma_start(
                out=wh_bf[:], in_=wh_d.rearrange("(fi p) u -> p fi u", p=128)
            )
            b_sb = const.tile([128, NU], FP32)
            nc.sync.dma_start(out=b_sb[:], in_=b_d.rearrange("(uj p) -> p uj", p=128))
            v_sb = const.tile([128, NU], FP32)
            nc.sync.dma_start(out=v_sb[:], in_=v_d.rearrange("(uj p) -> p uj", p=128))

            ident = const.tile([128, 128], FP32)
            make_identity(nc, ident[:])

            # h = X[:, -1, :] for all batches: [BPC, F] -> H.T tiles [128, NF, BPC]
            h_bf = const.tile([BPC, F], BF16)
            nc.gpsimd.dma_start(out=h_bf[:], in_=x_d[:, T - 1, :])
            ht = const.tile([128, NF, BPC], BF16)
            nc.sync.dma_start(out=ht[:], in_=h_bf[:], transpose=True)

            # bias_all[:, uj, b] = (h_b @ W_h + b)[uj*128 + p]
            bias_sb = const.tile([128, NU, BPC], FP32)
            for uj in range(NU):
                psh = pspool.tile([128, 512], FP32, tag="ps")
                for fi in range(NF):
                    nc.tensor.matmul(
                        psh[:, :BPC],
                        wh_bf[:, fi, uj * 128 : (uj + 1) * 128],
                        ht[:, fi, :],
                        start=(fi == 0),
                        stop=(fi == NF - 1),
                    )
                nc.scalar.activation(
                    bias_sb[:, uj, :], psh[:, :BPC], Ident, bias=b_sb[:, uj : uj + 1]
                )

            # ---- main loop over batches ----
            for bi in range(BPC):
                # natural layout bf16: xbf[p, ti, f] = X[ti*128+p, f]
                xbf = xpool.tile([128, NT, F], BF16)
                nc.gpsimd.dma_start(
                    out=xbf[:],
                    in_=x_d[bi].rearrange("(ti p) f -> p ti f", p=128),
                )
                # transposed: xt[p, ti, fi, q] = X[ti*128+q, fi*128+p]
                xt = xtpool.tile([128, NT, NF, 128], BF16)
                nc.sync.dma_start(out=xt[:], in_=xbf[:], transpose=True)

                score_sb = []
                for uj in range(NU):
                    s_sb = scpool.tile([128, T], FP32, tag="score")
                    score_sb.append(s_sb)
                    for th in range(2):
                        ps = pspool.tile([128, 512], FP32, tag="ps")
                        for fi in range(NF):
                            nc.tensor.matmul(
                                ps[:],
                                wo_bf[:, fi, uj * 128 : (uj + 1) * 128],
                                xt[:, th * 4 : (th + 1) * 4, fi, :],
                                start=(fi == 0),
                                stop=(fi == NF - 1),
                            )
                        nc.scalar.activation(
                            s_sb[:, th * 512 : (th + 1) * 512],
                            ps[:],
                            Tanh,
                            bias=bias_sb[:, uj, bi : bi + 1],
                        )

                # logits[t] in [128t, NT] layout
                logit_ps = psmall.tile([128, NT], FP32, tag="logit")
                for ti in range(NT):
                    for uj in range(NU):
                        nc.tensor.matmul(
                            logit_ps[:, ti : ti + 1],
                            score_sb[uj][:, ti * 128 : (ti + 1) * 128],
                            v_sb[:, uj : uj + 1],
                            start=(uj == 0),
                            stop=(uj == NU - 1),
                        )

                e_f = smsb.tile([128, NT], FP32, tag="ef")
                nc.scalar.activation(e_f[:], logit_ps[:], Exp)
                e_bf = smsb.tile([128, NT], BF16, tag="ebf")
                nc.vector.tensor_copy(e_bf[:], e_f[:])

                # transpose e -> [NT, 128] rows for contiguous output
                erow_ps = psmall.tile([NT, 128], FP32, tag="erow")
                nc.tensor.transpose(erow_ps[:], e_f[:], ident[:])
                erow = smsb.tile([NT, 128], FP32, tag="erowsb")
                nc.vector.tensor_copy(erow[:], erow_ps[:])
                nc.sync.dma_start(
                    out=e_d[bi].rearrange("(ti p) -> ti p", ti=NT), in_=erow[:]
                )

                # ctx_unnorm = e.T @ X  (contract over t)
                ctx_ps = psmall.tile([1, F], FP32, tag="ctx")
                for ti in range(NT):
                    nc.tensor.matmul(
                        ctx_ps[:],
                        e_bf[:, ti : ti + 1],
                        xbf[:, ti, :],
                        start=(ti == 0),
                        stop=(ti == NT - 1),
                    )
                ctx_sb = smsb.tile([1, F], FP32, tag="ctxsb")
                nc.vector.tensor_copy(ctx_sb[:], ctx_ps[:])
                nc.sync.dma_start(out=ctx_d[bi], in_=ctx_sb[:])

    nc.compile()
    return nc


_NC = None


def _get_nc():
    global _NC
    if _NC is None:
        _NC = build_nc()
    return _NC


def kernel(inputs, W_o, W_h, b, v):
    x = np.ascontiguousarray(np.asarray(inputs, dtype=np.float32))
    W_o = np.ascontiguousarray(np.asarray(W_o, dtype=np.float32))
    W_h = np.ascontiguousarray(np.asarray(W_h, dtype=np.float32))
    b = np.ascontiguousarray(np.asarray(b, dtype=np.float32))
    v = np.ascontiguousarray(np.asarray(v, dtype=np.float32))

    nc = _get_nc()
    in_maps = [
        {
            "x": x[c * BPC : (c + 1) * BPC],
            "w_o": W_o,
            "w_h": W_h,
            "b": b,
            "v": v,
        }
        for c in range(N_CORES)
    ]
    res = run_bass_kernel_spmd(nc, in_maps, list(range(N_CORES)))
    e = np.concatenate([res.results[c]["e_out"] for c in range(N_CORES)], axis=0)
    ctx = np.concatenate([res.results[c]["ctx_out"] for c in range(N_CORES)], axis=0)

    s = e.sum(axis=1, dtype=np.float64)
    attention_weights = (e / s[:, None]).astype(np.float32)
    context_vector = (ctx / s[:, None]).astype(np.float32)
    return (context_vector, attention_weights)
